# revision 1
# baseline (speedup 1.0000x reference)
"""CIN (Compressed Interaction Network) Trainium2 kernel.

Shapes (hardcoded from the problem spec):
  inputs (1024, 32, 16) f32; W0 (1024,128); W1/W2 (4096,128); b0/b1/b2 (128,)
  output (1024, 384) f32.

Strategy: pure data parallel over batch (8 cores x 128 rows).
Per core, everything lives in a transposed layout (features on partitions,
r = (b, d) with d innermost on the free dim, R = 128*16 = 2048):

  x0T[m, r]          = inputs[b, m, d]
  layer GEMMs accumulate in PSUM over K-chunks of the outer-product matrix
  Z^T[(h, m), r] = xlT[h, r] * x0T[m, r], with chunk partition layout
  p = m_loc*32 + h_loc covering (h = g*32 + h_loc, m = q*4 + m_loc).
  Weight chunks are host-permuted to match: Wp[c=(q*G+g), p, o].

  Z chunks are built on VectorE as bf16 tensor_tensor multiplies between
  xlrep[:, g, :] (xlT rows replicated 4x across partitions) and
  x0bc32[:, q, :] (x0T rows replicated 32x across partitions), both
  produced by SBUF->SBUF DMAs.

  Layer outputs come back fp32 in PSUM; ScalarE applies bias+relu twice
  (fp32 copy for the d-sum output reduction, bf16 copy to feed the next
  layer). Final per-layer outputs are sum over d: VectorE tensor_reduce
  over the innermost 16 elements -> (128 o, 128 b) f32, DMA'd out.
"""

import sys

sys.path.insert(0, "/opt/trn_rl_repo")

import numpy as np
import ml_dtypes

import concourse.bass as bass
import concourse.mybir as mybir
from concourse.tile import TileContext
from concourse.bass_utils import run_bass_kernel_spmd
from bass_rust import ScopedClock

# ---------------------------------------------------------------------------
# Workaround: this walrus build rejects >1 sync-wait on the Tile tail Drain.
# Emit the tail-drain waits as standalone 1-wait NOPs on the sync engine.
_orig_drain_and_barrier = TileContext._drain_and_barrier


def _patched_drain_and_barrier(self, tick_clock, wait_clock):
    nc = self.nc
    probe = nc.sync.nop()
    wait_clock.add_sem_waits(probe.ins, ScopedClock({None: tick_clock.global_clock}))
    si = probe.ins.sync_info
    waits = list(si.on_wait) if si and si.on_wait else []
    si.on_wait = []
    assert self.sems is not None
    by_name = {h.name: h for h in self.sems.allocated().values()}
    for w in waits:
        h = by_name.get(w.ant_name)
        assert h is not None, f"no sem handle for {w.ant_name}"
        nop = nc.sync.nop()
        nop.wait_op(h, w.wait_value, "sem-ge")
    nc.sync.drain()
    nc.all_engine_barrier()
    popped = nc._tile_sem_poison_stack.pop()
    assert popped is self._sem_poison
    nc.clear_and_free_semaphores(list(self.sems.allocated().values()))
    nc.all_engine_barrier()


TileContext._drain_and_barrier = _patched_drain_and_barrier
# ---------------------------------------------------------------------------

BATCH, M, D = 1024, 32, 16
H = 128
NCORES = 8
BC = BATCH // NCORES  # 128 rows per core
R = BC * D  # 2048
NSLAB = 4  # 4 x 512 free-dim slabs per matmul group
SLAB = R // NSLAB

F32 = mybir.dt.float32
BF16 = mybir.dt.bfloat16
AF = mybir.ActivationFunctionType

_cached = {}

WAIT_CAP = 1  # this walrus build allows few sync-waits per instruction


def _split_excess_waits(nc, cap=WAIT_CAP):
    """Hoist waits beyond `cap` per instruction onto standalone same-engine
    NOPs inserted right before the instruction (engine streams are in-order,
    so semantics are identical)."""
    for bbh in nc.bb_map.values():
        insts = bbh.bb.instructions
        idx = 0
        while idx < len(insts):
            ins = insts[idx]
            si = ins.sync_info
            waits = list(si.on_wait) if si and si.on_wait else []
            if len(waits) > cap:
                si.on_wait = waits[-cap:]
                for w in waits[:-cap]:
                    nop = mybir.InstNoOp(
                        name=nc.get_next_instruction_name(), ins=[], outs=[])
                    nop.engine = ins.engine
                    nop.sync_info = mybir.SyncInfo(on_wait=[w], on_update=[])
                    try:
                        nop.debug = ins.debug
                    except Exception:
                        pass
                    nc.register_instruction(nop, overwrite=True)
                    insts.insert(idx, nop)
                    idx += 1
            idx += 1


def _wperm(W: np.ndarray, h: int) -> np.ndarray:
    """W (h*32, 128) with row index k = h_idx*32 + m -> chunk layout
    (c=(q*G+g), p=m_loc*32+h_loc, o) with h = g*32+h_loc, m = q*4+m_loc."""
    G = h // 32
    Wr = W.reshape(G, 32, 8, 4, H)  # [g, h_loc, q, m_loc, o]
    return np.ascontiguousarray(
        np.transpose(Wr, (2, 0, 3, 1, 4)).reshape(8 * G, 128, H)
    )


def _build_program():
    nc = bass.Bass("TRN2", target_bir_lowering=False, debug=False,
                   num_devices=NCORES)

    inp = nc.dram_tensor("inp", [BC, M, D], F32, kind="ExternalInput").ap()
    w0p = nc.dram_tensor("w0p", [8, 128, H], BF16, kind="ExternalInput").ap()
    w1p = nc.dram_tensor("w1p", [32, 128, H], BF16, kind="ExternalInput").ap()
    w2p = nc.dram_tensor("w2p", [32, 128, H], BF16, kind="ExternalInput").ap()
    b0d = nc.dram_tensor("b0", [H, 1], F32, kind="ExternalInput").ap()
    b1d = nc.dram_tensor("b1", [H, 1], F32, kind="ExternalInput").ap()
    b2d = nc.dram_tensor("b2", [H, 1], F32, kind="ExternalInput").ap()
    selbd = nc.dram_tensor("selb", [8, 32, 128], BF16, kind="ExternalInput").ap()
    selrd = nc.dram_tensor("selr", [128, 128], BF16, kind="ExternalInput").ap()
    out_d = [
        nc.dram_tensor(f"out{i}", [H, BC], F32, kind="ExternalOutput").ap()
        for i in range(3)
    ]

    with TileContext(nc) as tc:
        with (
            tc.tile_pool(name="const", bufs=1) as cpool,
            tc.tile_pool(name="work", bufs=3) as wpool,
            tc.tile_pool(name="acts", bufs=2) as apool,
            tc.tile_pool(name="psum", bufs=2, space="PSUM") as ppool,
        ):
            # ---- load + prep x0 (input split in halves to cut latency) ----
            x0t_f = cpool.tile([M, BC, D], F32, name="x0t_f")
            x0t = cpool.tile([M, BC, D], BF16, name="x0t")
            HB = BC // 2
            for hh in range(2):
                bsl = slice(hh * HB, (hh + 1) * HB)
                nc.sync.dma_start(out=x0t_f[:, bsl, :],
                                  in_=inp.rearrange("b m d -> m b d")[:, bsl, :])
                nc.vector.tensor_copy(x0t[:, bsl, :], x0t_f[:, bsl, :])
            x0t = x0t.rearrange("m b d -> m (b d)")

            # selectors first: the broadcast matmuls need them immediately
            selb = cpool.tile([32, 8, 128], BF16, name="selb")
            nc.sync.dma_start(out=selb[:], in_=selbd.rearrange("q k p -> k q p"))
            selr = cpool.tile([128, 128], BF16, name="selr")
            nc.sync.dma_start(out=selr[:], in_=selrd)
            # x0rep4[p, r] = x0t[p % 32, r]
            x0rep4 = cpool.tile([128, R], BF16, name="x0rep4")
            for j in range(4):
                nc.sync.dma_start(out=x0rep4[j * 32:(j + 1) * 32, :], in_=x0t[:])
            x0bc32 = cpool.tile([128, 8, R], BF16, name="x0bc32")
            RH = R // 2
            for q in range(8):
                for hh in range(2):
                    bps = ppool.tile([128, RH], F32, name=f"bps{q}_{hh}",
                                     tag="bps", bufs=2)
                    for n in range(2):
                        sl = slice(hh * RH + n * SLAB, hh * RH + (n + 1) * SLAB)
                        nc.tensor.matmul(bps[:, n * SLAB:(n + 1) * SLAB],
                                         selb[:, q, :], x0t[:, sl],
                                         start=True, stop=True)
                    nc.scalar.activation(
                        x0bc32[:, q, hh * RH:(hh + 1) * RH], bps[:], AF.Copy)

            # ---- layer-0 weights + bias (later layers load during L0) ----
            w0s = cpool.tile([128, 8, H], BF16, name="w0s")
            nc.gpsimd.dma_start(out=w0s[:], in_=w0p.rearrange("c p o -> p c o"))
            bias = []
            for i, bd in enumerate((b0d, b1d, b2d)):
                bt = cpool.tile([H, 1], F32, name=f"bias{i}")
                nc.gpsimd.dma_start(out=bt[:], in_=bd)
                bias.append(bt)

            QB = 2  # q's per batched tensor_tensor multiply

            RH = R // 2  # layers are processed in independent r-halves
            BH = BC // 2

            def half_layer(li, hh, in0_of_g, nq, ng, wtile, btile, relu):
                """One r-half of one layer: Z-chunk TTs + accumulating GEMM,
                then bias(+relu) -> bf16 acts, d-sum output, and the next
                layer's replicated activations.  Halves share only read-only
                operands, so they pipeline freely."""
                rs = slice(hh * RH, (hh + 1) * RH)
                ps = ppool.tile([128, RH], F32, name=f"ps{li}_{hh}", tag="ps",
                                bufs=2)
                nchunks = nq * ng
                emitted = 0
                for g in range(ng):
                    for q0 in range(0, nq, QB):
                        zb = wpool.tile([128, QB, RH], BF16,
                                        name=f"zb{li}_{hh}_{g}_{q0}", tag="zb")
                        nc.vector.tensor_mul(
                            zb[:],
                            in0_of_g(g).unsqueeze(1).broadcast_to(
                                (128, QB, RH)),
                            x0bc32[:, q0:q0 + QB, rs],
                        )
                        for ql in range(QB):
                            c = (q0 + ql) * ng + g
                            for n in range(RH // SLAB):
                                sl = slice(n * SLAB, (n + 1) * SLAB)
                                nc.tensor.matmul(
                                    ps[:, sl], wtile[:, c, :], zb[:, ql, sl],
                                    start=(emitted == 0),
                                    stop=(emitted == nchunks - 1),
                                )
                            emitted += 1
                bsl = slice(hh * BH, (hh + 1) * BH)
                red = apool.tile([128, BH], F32, name=f"red{li}_{hh}",
                                 tag="redh", bufs=2)
                if relu:
                    act_b = apool.tile([128, RH], BF16, name=f"actb{li}_{hh}",
                                       tag="actb", bufs=2)
                    nc.scalar.activation(act_b[:], ps[:], AF.Relu,
                                         bias=btile[:])
                    xlrep_h = apool.tile([128, 4, RH], BF16,
                                         name=f"xlrep{li}_{hh}", tag="xlrep",
                                         bufs=2)
                    for g in range(4):
                        rps = ppool.tile([128, RH], F32,
                                         name=f"rps{li}{g}{hh}", tag="bps",
                                         bufs=2)
                        for n in range(RH // SLAB):
                            sl = slice(n * SLAB, (n + 1) * SLAB)
                            nc.tensor.matmul(
                                rps[:, sl],
                                selr[g * 32:(g + 1) * 32, :],
                                act_b[g * 32:(g + 1) * 32, sl],
                                start=True, stop=True,
                                tile_position=(g * 32, 0))
                        nc.scalar.activation(xlrep_h[:, g, :], rps[:], AF.Copy)
                    nc.vector.tensor_reduce(
                        red[:], act_b[:].rearrange("p (b d) -> p b d", d=D),
                        mybir.AxisListType.X, mybir.AluOpType.add,
                    )
                    nc.sync.dma_start(out=out_d[li][:, bsl], in_=red[:])
                    return xlrep_h
                # last layer: no relu -> sum psum over d, then add D*bias
                nc.vector.tensor_reduce(
                    red[:], ps[:].rearrange("p (b d) -> p b d", d=D),
                    mybir.AxisListType.X, mybir.AluOpType.add,
                )
                red2 = apool.tile([128, BH], F32, name=f"red2_{hh}",
                                  tag="red2h", bufs=2)
                nc.vector.tensor_scalar_add(red2[:], red[:], btile[:])
                nc.sync.dma_start(out=out_d[li][:, bsl], in_=red2[:])
                return None

            w1s = cpool.tile([128, 32, H], BF16, name="w1s")
            w2s = cpool.tile([128, 32, H], BF16, name="w2s")

            xr0 = {}
            xr0[0] = half_layer(0, 0, lambda g: x0rep4[:, 0:RH], 8, 1, w0s,
                                bias[0], True)
            # load later-layer weights while L0 computes
            nc.gpsimd.dma_start(out=w1s[:], in_=w1p.rearrange("c p o -> p c o"))
            nc.gpsimd.dma_start(out=w2s[:], in_=w2p.rearrange("c p o -> p c o"))
            xr0[1] = half_layer(0, 1, lambda g: x0rep4[:, RH:R], 8, 1, w0s,
                                bias[0], True)
            xr1 = {}
            for hh in range(2):
                xr1[hh] = half_layer(1, hh,
                                     lambda g, _x=xr0[hh]: _x[:, g, :],
                                     8, 4, w1s, bias[1], True)
            for hh in range(2):
                half_layer(2, hh, lambda g, _x=xr1[hh]: _x[:, g, :],
                           8, 4, w2s, bias[2], False)

    _split_excess_waits(nc)
    return nc


def _get_program():
    if "nc" not in _cached:
        _cached["nc"] = _build_program()
    return _cached["nc"]


def kernel(inputs, W0, b0, W1, b1, W2, b2, _want_trace=False):
    nc = _get_program()

    w0p = _wperm(np.asarray(W0, np.float32), 32).astype(ml_dtypes.bfloat16)
    w1p = _wperm(np.asarray(W1, np.float32), 128).astype(ml_dtypes.bfloat16)
    w2p = _wperm(np.asarray(W2, np.float32), 128).astype(ml_dtypes.bfloat16)
    selb = np.zeros((8, 32, 128), ml_dtypes.bfloat16)
    for q in range(8):
        for p in range(128):
            selb[q, q * 4 + p // 32, p] = 1
    selr = np.zeros((128, 128), ml_dtypes.bfloat16)
    for k in range(128):
        for p in range(128):
            if k % 32 == p % 32:
                selr[k, p] = 1
    shared = {
        "w0p": w0p, "w1p": w1p, "w2p": w2p, "selb": selb, "selr": selr,
        "b0": np.asarray(b0, np.float32).reshape(H, 1),
        "b1": np.asarray(b1, np.float32).reshape(H, 1),
        "b2": np.asarray(b2, np.float32).reshape(H, 1) * D,
    }
    inputs = np.ascontiguousarray(np.asarray(inputs, np.float32))
    in_maps = [
        {"inp": inputs[c * BC:(c + 1) * BC], **shared} for c in range(NCORES)
    ]
    res = run_bass_kernel_spmd(nc, in_maps, list(range(NCORES)),
                               trace=_want_trace)
    out = np.empty((BATCH, 3 * H), np.float32)
    for c in range(NCORES):
        r = res.results[c]
        for i in range(3):
            out[c * BC:(c + 1) * BC, i * H:(i + 1) * H] = r[f"out{i}"].T
    if _want_trace:
        return out, res
    return out

